# revision 1
# baseline (speedup 1.0000x reference)
"""Self-contained Trainium2 Bass kernel for the MGNN heterogeneous GNN.

Computation (see reference): row-normalize 38 adjacency matrices, group-sum
them, and run 2 layers of message passing + 3-view semantic attention + fc.

Key algebraic identities used:
  * diag(1/r) A @ X == diag(1/r) (A @ X)  -> normalize AFTER the matmul
  * rowsum(A_i) == (A_i @ [X | 1])[:, -1] -> rowsums come free from a ones col
so the raw adjacencies are streamed through the TensorEngine exactly once per
layer, with no separate normalization pass.

Sharding: row-parallel over destination nodes across 8 cores (each core owns
ND/8 drug rows + NP/8 protein rows of every adjacency).  Updated node
features are AllGathered (bf16) between layers.

Per-tile pipeline: DMA fp32 row-slab -> PE transpose (identity matmul) ->
PSUM -> copy-cast to bf16 SBUF (DVE/ACT alternating) -> bf16 matmul with
[X|1] -> per-row 1/rowsum scale on ACT -> accumulate in SBUF fp32.
"""

import numpy as np
from contextlib import ExitStack

import concourse.bass as bass
import concourse.mybir as mybir
import concourse.tile as tile
from concourse import bacc
from concourse.bass_utils import run_bass_kernel_spmd
from concourse.masks import make_identity

F32 = mybir.dt.float32
BF16 = mybir.dt.bfloat16
AF = mybir.ActivationFunctionType
ALU = mybir.AluOpType


class MGNNConfig:
    def __init__(
        self,
        ND=2048, NP=4096, DIM=256, D_IN=1024, P_IN=1024, ATT_H=128,
        n_dd=9, n_pd=10, n_dp=10, n_pp=9,
        # per z-view (v0, v1, v2) index ranges into each adjacency stack
        dd_slices=((8, 9), (0, 5), (5, 8)),
        pd_slices=((6, 10), (0, 4), (4, 6)),
        dp_slices=((6, 10), (0, 4), (4, 6)),
        pp_slices=((8, 9), (0, 5), (5, 8)),
        w_idx=(2, 0, 1),  # relation-weight index per z-view
        n_layers=2, n_cores=8, NDS=None, NPS=None,
    ):
        self.ND, self.NP, self.DIM = ND, NP, DIM
        self.D_IN, self.P_IN, self.ATT_H = D_IN, P_IN, ATT_H
        self.n_dd, self.n_pd, self.n_dp, self.n_pp = n_dd, n_pd, n_dp, n_pp
        self.dd_slices, self.pd_slices = dd_slices, pd_slices
        self.dp_slices, self.pp_slices = dp_slices, pp_slices
        self.w_idx = w_idx
        self.n_layers, self.n_cores = n_layers, n_cores
        self.NDS = NDS or ND // n_cores   # drug rows per core
        self.NPS = NPS or NP // n_cores   # protein rows per core
        assert ND % (128 * n_cores) == 0 and NP % (128 * n_cores) == 0
        assert DIM == 256 and ATT_H == 128


def emit_mgnn(tc, cfg, ins, outs):
    """Emit the whole model into an open TileContext.

    ins/outs: dicts of bass.AP for the per-core DRAM tensors.
    """
    ctx = ExitStack()
    nc = tc.nc
    DIM = cfg.DIM
    NMB_D = cfg.NDS // 128     # drug dest row-blocks per core
    NMB_P = cfg.NPS // 128     # protein dest row-blocks per core
    KB_D = cfg.ND // 128       # source k-blocks, drug-source relations
    KB_P = cfg.NP // 128

    with ctx:
        # ---------------- pools ----------------
        const = ctx.enter_context(tc.tile_pool(name="const", bufs=1))
        dram = ctx.enter_context(tc.tile_pool(name="dram", bufs=1, space="DRAM"))

        ps_trps = ctx.enter_context(tc.tile_pool(name="ps_trps", bufs=1, space="PSUM"))
        ps_trpb = ctx.enter_context(tc.tile_pool(name="ps_trpb", bufs=3, space="PSUM"))
        ps_msg = ctx.enter_context(tc.tile_pool(name="ps_msg", bufs=2, space="PSUM"))
        ps_proj = ctx.enter_context(tc.tile_pool(name="ps_proj", bufs=2, space="PSUM"))

        # ---------------- constants ----------------
        ident = const.tile([128, 128], F32, tag="ident")
        make_identity(nc, ident)
        ident_bf = const.tile([128, 128], BF16, tag="ident_bf")
        nc.vector.tensor_copy(ident_bf, ident)

        cnt = [0]  # alternate DVE/ACT for big PSUM->SBUF copies

        def copycast(dst, src):
            cnt[0] += 1
            if cnt[0] % 5 < 3:
                nc.vector.tensor_copy(dst, src)
            else:
                nc.scalar.copy(dst, src)

        # PE transpose of bf16 natural [128, 128*n] -> bf16 [128, n, 128]
        def transpose_bf(nat_ap, nblk, dst, dst_pref):
            """nat_ap: [128, 128*nblk] bf16 AP; writes dst[:, dst_pref+j, :] bf16."""
            for t0 in range(0, nblk, 4):
                tw = min(4, nblk - t0)
                trb = ps_trpb.tile([128, 4, 128], BF16, tag="trpb")
                for j in range(tw):
                    t = t0 + j
                    nc.tensor.transpose(
                        trb[:, j, :], nat_ap[:, t * 128:(t + 1) * 128], ident_bf
                    )
                copycast(dst[:, dst_pref + t0:dst_pref + t0 + tw, :], trb[:, :tw, :])

        # ---------------- AllGather bounce buffers ----------------
        # A Shared DRAM tensor may only be written once, so each layer
        # boundary gets its own buffer set.
        ag_space = "Shared" if cfg.n_cores > 4 else "Local"
        ag_bufs = []
        for b in range(cfg.n_layers):
            ag_bufs.append((
                dram.tile([cfg.NDS, DIM], BF16, tag=f"ag_d_in{b}", name=f"ag_d_in{b}"),
                dram.tile([cfg.ND, DIM], BF16, tag=f"ag_d_out{b}", name=f"ag_d_out{b}",
                          addr_space=ag_space),
                dram.tile([cfg.NPS, DIM], BF16, tag=f"ag_p_in{b}", name=f"ag_p_in{b}"),
                dram.tile([cfg.NP, DIM], BF16, tag=f"ag_p_out{b}", name=f"ag_p_out{b}",
                          addr_space=ag_space),
            ))

        rgroups = [list(range(cfg.n_cores))]

        # layer-invariant transposed-adjacency spill stores (bf16)
        at_store = {}
        if cfg.n_layers > 1:
            for rel, n_i, nmb, kbn in (
                ("dd", cfg.n_dd, NMB_D, KB_D), ("pd", cfg.n_pd, NMB_D, KB_P),
                ("dp", cfg.n_dp, NMB_P, KB_D), ("pp", cfg.n_pp, NMB_P, KB_P),
            ):
                at_store[rel] = dram.tile(
                    [n_i, nmb, 128, kbn * 128], BF16,
                    tag=f"at_{rel}", name=f"at_{rel}",
                )

        # ------- weights + initial projection in a transient SBUF pool -------
        w_rel = {}
        with tc.tile_pool(name="wload", bufs=1) as wload:

            def load_weight_bf16(tag, w_ap):
                """DRAM fp32 [K, W] -> SBUF bf16 tile [128, K//128, W]."""
                K, W = w_ap.shape
                kb = K // 128
                stage = wload.tile([128, kb, W], F32, tag="wstage", bufs=2)
                nc.sync.dma_start(
                    out=stage, in_=w_ap.rearrange("(b p) w -> p b w", p=128)
                )
                wt = const.tile([128, kb, W], BF16, tag=tag)
                nc.vector.tensor_copy(wt, stage)
                return wt

            w_proj_d = load_weight_bf16("w_proj_d", ins["Wproj_d"])
            w_proj_p = load_weight_bf16("w_proj_p", ins["Wproj_p"])
            for rel in ("dd", "pd", "dp", "pp"):
                for g in range(3):
                    w_rel[(rel, g)] = load_weight_bf16(f"w_{rel}{g}", ins[f"W{rel}"][g])
            w_fc_d = load_weight_bf16("w_fc_d", ins["Wfc_d"])
            w_fc_p = load_weight_bf16("w_fc_p", ins["Wfc_p"])
            w_a1 = load_weight_bf16("w_a1", ins["Wa1"])
            w_a2 = load_weight_bf16("w_a2", ins["Wa2"])

            # initial projection: d0 = drug_feat @ Wproj_d (bf16) -> AG input
            def initial_proj(feat_ap, FIN, nmb, w_proj, ag_in):
                fkb = FIN // 128
                for nb in range(nmb):
                    nat = wload.tile([128, FIN], BF16, tag="nat_feat", bufs=2)
                    nc.gpsimd.dma_start(out=nat, in_=feat_ap[nb * 128:(nb + 1) * 128, :])
                    featT = wload.tile([128, fkb, 128], BF16, tag="featT", bufs=2)
                    transpose_bf(nat, fkb, featT, 0)
                    ps = ps_proj.tile([128, 512], F32, tag="proj")
                    for fb in range(fkb):
                        nc.tensor.matmul(
                            ps[:, :DIM], lhsT=featT[:, fb, :], rhs=w_proj[:, fb, :],
                            start=(fb == 0), stop=(fb == fkb - 1),
                        )
                    ot = wload.tile([128, DIM], BF16, tag="proj_out0", bufs=3)
                    copycast(ot, ps[:, :DIM])
                    nc.sync.dma_start(out=ag_in[nb * 128:(nb + 1) * 128, :], in_=ot)

            initial_proj(ins["drug_feat_sh"], cfg.D_IN, NMB_D, w_proj_d, ag_bufs[0][0])
            initial_proj(ins["protein_feat_sh"], cfg.P_IN, NMB_P, w_proj_p, ag_bufs[0][2])

        # ---------------- main pools (after transient pool freed) ----------------
        natA = ctx.enter_context(tc.tile_pool(name="natA", bufs=2))
        atbp = ctx.enter_context(tc.tile_pool(name="atbp", bufs=2))
        natx = ctx.enter_context(tc.tile_pool(name="natx", bufs=3))
        xap = ctx.enter_context(tc.tile_pool(name="xap", bufs=1))
        hp = ctx.enter_context(tc.tile_pool(name="hp", bufs=1))
        misc = ctx.enter_context(tc.tile_pool(name="misc", bufs=2))

        def allgather(b):
            di, do, pi, po = ag_bufs[b]
            if cfg.n_cores == 1:
                # timing-model stand-in (single core has nothing to gather)
                nc.sync.dma_start(out=do[:cfg.NDS, :], in_=di[:, :])
                nc.sync.dma_start(out=po[:cfg.NPS, :], in_=pi[:, :])
                return
            nc.gpsimd.collective_compute(
                "AllGather", ALU.bypass, replica_groups=rgroups,
                ins=[di.opt()], outs=[do.opt()],
            )
            nc.gpsimd.collective_compute(
                "AllGather", ALU.bypass, replica_groups=rgroups,
                ins=[pi.opt()], outs=[po.opt()],
            )

        allgather(0)

        # ================ layers ================
        for layer in range(cfg.n_layers):
            last = layer == cfg.n_layers - 1

            # ---- transpose full d/p (bf16) into lhsT tiles ----
            dT = misc.tile([128, 2, KB_D, 128], BF16, tag="dT", bufs=1)
            pT = misc.tile([128, 2, KB_P, 128], BF16, tag="pT", bufs=1)
            for (src, kb_n, dstT) in ((ag_bufs[layer][1], KB_D, dT), (ag_bufs[layer][3], KB_P, pT)):
                for nb in range(kb_n):
                    nx = natx.tile([128, DIM], BF16, tag="natx")
                    nc.sync.dma_start(out=nx, in_=src[nb * 128:(nb + 1) * 128, :])
                    trb = ps_trpb.tile([128, 4, 128], BF16, tag="trpb")
                    for fb in range(2):
                        nc.tensor.transpose(
                            trb[:, fb, :], nx[:, fb * 128:(fb + 1) * 128], ident_bf
                        )
                    copycast(dstT[:, :, nb, :], trb[:, :2, :])

            # h accumulators [128, 3 * nmb, DIM] fp32 (view-major slots)
            h_d = hp.tile([128, 3 * NMB_D, DIM], F32, tag="h_d")
            h_p = hp.tile([128, 3 * NMB_P, DIM], F32, tag="h_p")

            # projected features with ones column:  xa[:, kb, 0:DIM]=X, [DIM]=1
            def project(tag, srcT, kb_n, wt):
                xa = xap.tile([128, kb_n, DIM + 1], BF16, tag=tag, bufs=2)
                for nb in range(kb_n):
                    ps = ps_proj.tile([128, 512], F32, tag="proj")
                    for fb in range(2):
                        nc.tensor.matmul(
                            ps[:, :DIM], lhsT=srcT[:, fb, nb, :], rhs=wt[:, fb, :],
                            start=(fb == 0), stop=(fb == 1),
                        )
                    copycast(xa[:, nb, :DIM], ps[:, :DIM])
                nc.vector.memset(xa[:, :, DIM:DIM + 1], 1.0)
                return xa

            # ---- message passing for one destination row-block ----
            def msg_rows(h_slice, pieces, first):
                # pieces: list of (rel, A_dram, i_lo, i_hi, mb, kb_n, xa)
                for (rel, A, i_lo, i_hi, mb, kb_n, xa) in pieces:
                    for i in range(i_lo, i_hi):
                        pm = ps_msg.tile([128, DIM + 1], F32, tag="pm")
                        spill = at_store.get(rel) if layer == 0 else None
                        if layer > 0:
                            # reload the bf16 transposed slab spilled in layer 0
                            slab = natA.tile([128, KB_P * 128], BF16, tag="natA", bufs=3)
                            nc.sync.dma_start(
                                out=slab[:, :kb_n * 128],
                                in_=at_store[rel][i, mb, :, :],
                            )
                            for t in range(kb_n):
                                nc.tensor.matmul(
                                    pm, lhsT=slab[:, t * 128:(t + 1) * 128],
                                    rhs=xa[:, t, :],
                                    start=(t == 0), stop=(t == kb_n - 1),
                                )
                        else:
                            nat = natA.tile([128, KB_P * 128], BF16, tag="natA", bufs=3)
                            nc.gpsimd.dma_start(
                                out=nat[:, :kb_n * 128],
                                in_=A[i, mb * 128:(mb + 1) * 128, :],
                            )
                            tsl = atbp.tile([128, KB_P * 128], BF16, tag="tslab", bufs=3)
                            for t0 in range(0, kb_n, 4):
                                tw = min(4, kb_n - t0)
                                trb = ps_trpb.tile([128, 4, 128], BF16, tag="trpb")
                                for j in range(tw):
                                    t = t0 + j
                                    nc.tensor.transpose(
                                        trb[:, j, :], nat[:, t * 128:(t + 1) * 128],
                                        ident_bf,
                                    )
                                copycast(
                                    tsl[:, t0 * 128:(t0 + tw) * 128], trb[:, :tw, :]
                                )
                                for j in range(tw):
                                    t = t0 + j
                                    nc.tensor.matmul(
                                        pm, lhsT=tsl[:, t * 128:(t + 1) * 128],
                                        rhs=xa[:, t, :],
                                        start=(t == 0), stop=(t == kb_n - 1),
                                    )
                            if spill is not None:
                                nc.sync.dma_start(
                                    out=spill[i, mb, :, :kb_n * 128],
                                    in_=tsl[:, :kb_n * 128],
                                )
                        # row scale = 1 / max(rowsum, eps);  zero rows stay zero
                        rs = misc.tile([128, 1], F32, tag="rs", bufs=3)
                        nc.vector.tensor_scalar_max(rs, pm[:, DIM:DIM + 1], 1e-30)
                        rinv = misc.tile([128, 1], F32, tag="rinv", bufs=3)
                        nc.vector.reciprocal(rinv, rs)
                        if first:
                            nc.scalar.activation(h_slice, pm[:, :DIM], AF.Copy, scale=rinv)
                            first = False
                        else:
                            tmp = misc.tile([128, DIM], F32, tag="msgtmp", bufs=3)
                            nc.scalar.activation(tmp, pm[:, :DIM], AF.Copy, scale=rinv)
                            nc.vector.tensor_add(h_slice, h_slice, tmp)
                nc.scalar.activation(h_slice, h_slice, AF.Relu)

            for v in range(3):
                w = cfg.w_idx[v]
                xa_pd = project("xa_dsrc", dT, KB_D, w_rel[("dd", w)])
                xa_ppd = project("xa_psrc", pT, KB_P, w_rel[("pd", w)])
                for mb in range(NMB_D):
                    msg_rows(
                        h_d[:, v * NMB_D + mb, :],
                        [
                            ("dd", ins["A_dd_sh"], *cfg.dd_slices[v], mb, KB_D, xa_pd),
                            ("pd", ins["A_pd_sh"], *cfg.pd_slices[v], mb, KB_P, xa_ppd),
                        ],
                        True,
                    )
                xa_pdp = project("xa_dsrc", dT, KB_D, w_rel[("dp", w)])
                xa_ppp = project("xa_psrc", pT, KB_P, w_rel[("pp", w)])
                for mb in range(NMB_P):
                    msg_rows(
                        h_p[:, v * NMB_P + mb, :],
                        [
                            ("dp", ins["A_dp_sh"], *cfg.dp_slices[v], mb, KB_D, xa_pdp),
                            ("pp", ins["A_pp_sh"], *cfg.pp_slices[v], mb, KB_P, xa_ppp),
                        ],
                        True,
                    )

            # ---- attention over the 3 views + fc ----
            def attention_fc(h_all, nmb, w_fc, ag_in, out_ext):
                n_nodes = nmb * 128
                assert n_nodes <= 512
                # h^T (bf16) for the score matmuls
                hT = misc.tile([128, 3, 2, n_nodes], BF16, tag="hT", bufs=1)
                for v in range(3):
                    for mb in range(nmb):
                        trp = ps_trps.tile([128, 4, 128], F32, tag="trps")
                        for fb in range(2):
                            nc.tensor.transpose(
                                trp[:, fb, :],
                                h_all[:, v * nmb + mb, fb * 128:(fb + 1) * 128],
                                ident,
                            )
                        copycast(hT[:, v, :, mb * 128:(mb + 1) * 128], trp[:, :2, :])
                # scores w_v = relu(h_v @ Wa1) @ Wa2; kept on one partition as
                # [1, 3, n_nodes] so softmax slices stay at base partition 0.
                w3 = misc.tile([1, 3, n_nodes], F32, tag="w3", bufs=1)
                for v in range(3):
                    ps_s = ps_proj.tile([128, 512], F32, tag="proj")
                    for fb in range(2):
                        nc.tensor.matmul(
                            ps_s[:, :n_nodes], lhsT=w_a1[:, fb, :], rhs=hT[:, v, fb, :],
                            start=(fb == 0), stop=(fb == 1),
                        )
                    s1 = misc.tile([128, n_nodes], BF16, tag="s1", bufs=2)
                    nc.scalar.activation(s1, ps_s[:, :n_nodes], AF.Relu)
                    ps_w = ps_proj.tile([1, 512], F32, tag="proj")
                    nc.tensor.matmul(
                        ps_w[:, :n_nodes], lhsT=w_a2[:, 0, :], rhs=s1,
                        start=True, stop=True,
                    )
                    nc.vector.tensor_copy(w3[:, v, :], ps_w[:, :n_nodes])
                # softmax over the 3 views (stable)
                mx = misc.tile([1, n_nodes], F32, tag="mx", bufs=1)
                nc.vector.tensor_tensor(mx, w3[:, 0, :], w3[:, 1, :], op=ALU.max)
                nc.vector.tensor_tensor(mx, mx, w3[:, 2, :], op=ALU.max)
                e3 = misc.tile([1, 3, n_nodes], F32, tag="e3", bufs=1)
                for v in range(3):
                    nc.vector.tensor_tensor(e3[:, v, :], w3[:, v, :], mx, op=ALU.subtract)
                nc.scalar.activation(e3, e3, AF.Exp)
                ssum = misc.tile([1, n_nodes], F32, tag="ssum", bufs=1)
                nc.vector.tensor_add(ssum, e3[:, 0, :], e3[:, 1, :])
                nc.vector.tensor_add(ssum, ssum, e3[:, 2, :])
                srec = misc.tile([1, n_nodes], F32, tag="srec", bufs=1)
                nc.vector.reciprocal(srec, ssum)
                for v in range(3):
                    nc.vector.tensor_tensor(e3[:, v, :], e3[:, v, :], srec, op=ALU.mult)
                # alpha -> natural layout [128, nmb, 3] via K=1 PE transposes
                al = misc.tile([128, nmb, 3], F32, tag="alpha", bufs=1)
                for mb in range(nmb):
                    trp = ps_trps.tile([128, 4, 128], F32, tag="trps")
                    for v in range(3):
                        nc.tensor.transpose(
                            trp[:, 0, v:v + 1],
                            e3[:, v, mb * 128:(mb + 1) * 128],
                            ident[:1, :1],
                        )
                    nc.vector.tensor_copy(al[:, mb, :], trp[:, 0, :3])
                # out rows = relu( (sum_v alpha_v * h_v) @ Wfc )
                for mb in range(nmb):
                    att = misc.tile([128, DIM], F32, tag="att", bufs=2)
                    tmp = misc.tile([128, DIM], F32, tag="att_tmp", bufs=2)
                    nc.vector.tensor_scalar_mul(att, h_all[:, 0 * nmb + mb, :], al[:, mb, 0:1])
                    for v in (1, 2):
                        nc.vector.tensor_scalar_mul(tmp, h_all[:, v * nmb + mb, :], al[:, mb, v:v + 1])
                        nc.vector.tensor_add(att, att, tmp)
                    trp = ps_trps.tile([128, 4, 128], F32, tag="trps")
                    for fb in range(2):
                        nc.tensor.transpose(trp[:, fb, :], att[:, fb * 128:(fb + 1) * 128], ident)
                    attT = misc.tile([128, 2, 128], BF16, tag="attT", bufs=2)
                    copycast(attT, trp[:, :2, :])
                    ps_fc = ps_proj.tile([128, 512], F32, tag="proj")
                    for fb in range(2):
                        nc.tensor.matmul(
                            ps_fc[:, :DIM], lhsT=attT[:, fb, :], rhs=w_fc[:, fb, :],
                            start=(fb == 0), stop=(fb == 1),
                        )
                    if out_ext is None:
                        ot = misc.tile([128, DIM], BF16, tag="proj_out", bufs=3)
                        nc.scalar.activation(ot, ps_fc[:, :DIM], AF.Relu)
                        nc.sync.dma_start(out=ag_in[mb * 128:(mb + 1) * 128, :], in_=ot)
                    else:
                        ot = misc.tile([128, DIM], F32, tag="out_f32", bufs=3)
                        nc.scalar.activation(ot, ps_fc[:, :DIM], AF.Relu)
                        nc.sync.dma_start(out=out_ext[mb * 128:(mb + 1) * 128, :], in_=ot)

            nxt = None if last else ag_bufs[layer + 1]
            attention_fc(h_d, NMB_D, w_fc_d, None if last else nxt[0],
                         outs["out_d"] if last else None)
            attention_fc(h_p, NMB_P, w_fc_p, None if last else nxt[2],
                         outs["out_p"] if last else None)

            if not last:
                allgather(layer + 1)


def build_nc(cfg, repeat=1):
    nc = bacc.Bacc(
        "TRN2", target_bir_lowering=False, debug=False,
        enable_asserts=False, num_devices=cfg.n_cores,
    )
    ins = {}

    def inp(name, shape):
        ins[name] = nc.dram_tensor(name, list(shape), F32, kind="ExternalInput").ap()

    inp("A_dd_sh", (cfg.n_dd, cfg.NDS, cfg.ND))
    inp("A_pd_sh", (cfg.n_pd, cfg.NDS, cfg.NP))
    inp("A_dp_sh", (cfg.n_dp, cfg.NPS, cfg.ND))
    inp("A_pp_sh", (cfg.n_pp, cfg.NPS, cfg.NP))
    inp("drug_feat_sh", (cfg.NDS, cfg.D_IN))
    inp("protein_feat_sh", (cfg.NPS, cfg.P_IN))
    inp("Wproj_d", (cfg.D_IN, cfg.DIM))
    inp("Wproj_p", (cfg.P_IN, cfg.DIM))
    for rel, n1, n2 in (("dd", cfg.DIM, cfg.DIM), ("pd", cfg.DIM, cfg.DIM),
                        ("dp", cfg.DIM, cfg.DIM), ("pp", cfg.DIM, cfg.DIM)):
        inp(f"W{rel}", (3, n1, n2))
    inp("Wfc_d", (cfg.DIM, cfg.DIM))
    inp("Wfc_p", (cfg.DIM, cfg.DIM))
    inp("Wa1", (cfg.DIM, cfg.ATT_H))
    inp("Wa2", (cfg.ATT_H, 1))

    outs = {
        "out_d": nc.dram_tensor("out_d", [cfg.NDS, cfg.DIM], F32, kind="ExternalOutput").ap(),
        "out_p": nc.dram_tensor("out_p", [cfg.NPS, cfg.DIM], F32, kind="ExternalOutput").ap(),
    }

    with tile.TileContext(nc) as tc:
        for _ in range(repeat):
            emit_mgnn(tc, cfg, ins, outs)
    nc.compile()
    return nc


def shard_inputs(cfg, inp):
    """Full-problem numpy inputs -> per-core in_maps."""
    R = cfg.n_cores
    f32 = lambda x: np.ascontiguousarray(np.asarray(x), dtype=np.float32)
    weights = {
        "Wproj_d": f32(inp["Wproj_d"]), "Wproj_p": f32(inp["Wproj_p"]),
        "Wdd": f32(inp["Wdd"]), "Wpd": f32(inp["Wpd"]),
        "Wdp": f32(inp["Wdp"]), "Wpp": f32(inp["Wpp"]),
        "Wfc_d": f32(inp["Wfc_d"]), "Wfc_p": f32(inp["Wfc_p"]),
        "Wa1": f32(inp["Wa1"]), "Wa2": f32(inp["Wa2"]),
    }
    A_dd = f32(inp["A_dd"]); A_pd = f32(inp["A_pd"])
    A_dp = f32(inp["A_dp"]); A_pp = f32(inp["A_pp"])
    df = f32(inp["drug_feat"]); pf = f32(inp["protein_feat"])
    in_maps = []
    for r in range(R):
        dsl = slice(r * cfg.NDS, (r + 1) * cfg.NDS)
        psl = slice(r * cfg.NPS, (r + 1) * cfg.NPS)
        m = {
            "A_dd_sh": np.ascontiguousarray(A_dd[:, dsl, :]),
            "A_pd_sh": np.ascontiguousarray(A_pd[:, dsl, :]),
            "A_dp_sh": np.ascontiguousarray(A_dp[:, psl, :]),
            "A_pp_sh": np.ascontiguousarray(A_pp[:, psl, :]),
            "drug_feat_sh": np.ascontiguousarray(df[dsl]),
            "protein_feat_sh": np.ascontiguousarray(pf[psl]),
        }
        m.update(weights)
        in_maps.append(m)
    return in_maps


_NC_CACHE = {}
TRACE = False          # set True (e.g. from a test harness) to capture an NTFF profile
LAST_RESULTS = None    # BassKernelResults of the most recent run


def kernel(**inputs):
    global LAST_RESULTS
    inp = {k: np.asarray(v) for k, v in inputs.items()}
    cfg = MGNNConfig(n_layers=int(inp.get("n_layers", 2)))
    key = cfg.n_layers
    if key not in _NC_CACHE:
        _NC_CACHE[key] = build_nc(cfg)
    nc = _NC_CACHE[key]
    in_maps = shard_inputs(cfg, inp)
    res = run_bass_kernel_spmd(
        nc, in_maps, core_ids=list(range(cfg.n_cores)), trace=TRACE
    )
    LAST_RESULTS = res
    outs = res.results
    d = np.concatenate([outs[r]["out_d"] for r in range(cfg.n_cores)], axis=0)
    p = np.concatenate([outs[r]["out_p"] for r in range(cfg.n_cores)], axis=0)
    return np.concatenate([d, p], axis=0).astype(np.float32)

